# revision 13
# baseline (speedup 1.0000x reference)
"""Trainium2 Bass kernel for softmax RGB blend (pytorch3d NoLightShader).

Full inputs (N=8, H=512, W=512, K=8) are sharded batch-wise across 8
NeuronCores (one batch image per core); the blend is purely per-pixel so no
cross-core communication is needed.

Math per pixel (K faces):
    mask_k  = pix_to_face_k >= 0
    prob_k  = sigmoid(-dists_k / SIGMA) * mask_k
    alpha   = 1 - prod_k(1 - prob_k)        (via exp(sum ln(1 - prob_k)))
    z_k     = (ZFAR - zbuf_k) / (ZFAR - ZNEAR) * mask_k
    zmax    = max_k z_k                     (EPS clamp dropped: only matters
                                             for all-masked pixels, where the
                                             result is unchanged)
    w_k     = prob_k * exp((z_k - zmax) / GAMMA)
    delta   = exp((EPS - zmax) / GAMMA)
    denom   = sum_k w_k + delta
    rgb_c   = (sum_k w_k * color_kc + delta) / denom    (background = 1,1,1)
    out     = [rgb, alpha]

Raw-bass pipeline (Tile's multi-wait instructions don't compile on this
walrus, so waits are explicit single-sem instructions):
    SP  (sync) : HWDGE DMAs in/out, double-buffered input slots
    ACT (scalar): sigmoid, z-linearize, ln(1-prob), exp(zd/g), delta, prod(q)
    DVE (vector): mask, mask applies, the four K-reductions, w, w*c,
                  denom, reciprocal, rgb/alpha finalize
Per-tile op indices give deterministic semaphore thresholds; see marks below.
"""

import sys
from contextlib import ExitStack

import numpy as np

if "/opt/trn_rl_repo" not in sys.path:
    sys.path.insert(0, "/opt/trn_rl_repo")

SIGMA = 1e-4
GAMMA = 1e-4
ZNEAR = 1.0
ZFAR = 100.0
EPS = 1e-10

P = 128
K = 8
N_CORES = 8

# per-tile op counts (sem increments per tile per engine)
N_ACT = 9   # sig, zlin, lnq, ex, delta, prodq, lnd, rcp, alpha
N_DVE = 12  # prob, zinv, zmax, qsum, zd, w, wc, wsum, csum, denom, t3, rgb


def build_program(rows, T):
    import concourse.bass as bass
    from concourse import mybir

    dt = mybir.dt
    f32 = dt.float32
    Alu = mybir.AluOpType
    Act = mybir.ActivationFunctionType
    Ax = mybir.AxisListType

    assert rows % T == 0
    n = rows // T
    TK = T * K

    nc = bass.Bass()

    zb_d = nc.dram_tensor("zbuf", [P, rows * K], f32, kind="ExternalInput")
    ds_d = nc.dram_tensor("dists", [P, rows * K], f32, kind="ExternalInput")
    pf_d = nc.dram_tensor("pix_to_face", [P, rows * K], dt.int32, kind="ExternalInput")
    pc_d = nc.dram_tensor("pixel_colors", [P, rows * K * 3], f32, kind="ExternalInput")
    out_d = nc.dram_tensor("out", [P, rows * 4], f32, kind="ExternalOutput")

    # const AP for the delta bias (EPS/GAMMA); framework pre-registers 0.0/1.0
    cbias = nc.alloc_sbuf_tensor("c_epsg", [P, 1], f32)
    nc.gpsimd.memset(cbias.ap(), EPS / GAMMA)
    nc.const_aps.aps[(f32, EPS / GAMMA)] = cbias.ap()
    nc.all_engine_barrier()

    # marks: value of the engine sem after op `idx` (1-based) of tile i
    am = lambda i, idx: i * N_ACT + idx
    dm = lambda i, idx: i * N_DVE + idx

    with ExitStack() as ctx:
        sb = lambda name, w: ctx.enter_context(nc.sbuf_tensor(name, [P, w], f32))
        # double-buffered input slots
        zb = [sb(f"zb{j}", TK) for j in range(2)]
        ds = [sb(f"ds{j}", TK) for j in range(2)]
        pf = [
            ctx.enter_context(nc.sbuf_tensor(f"pf{j}", [P, TK], dt.int32))
            for j in range(2)
        ]
        col = [sb(f"col{j}", TK * 3) for j in range(2)]
        ot = [sb(f"ot{j}", T * 4) for j in range(2)]
        # single-buffered intermediates (safety proven by the sem chains)
        sig = sb("sig", TK)      # becomes prob in place
        zlin = sb("zlin", TK)    # becomes zinv in place
        lnq = sb("lnq", TK)
        zd = sb("zd", TK)        # becomes ex in place
        w = sb("w", TK)
        wc = sb("wc", TK * 3)
        zmax = sb("zmax", T)
        qsum = sb("qsum", T)
        wsum = sb("wsum", T)
        csum = sb("csum", T * 3)
        delta = sb("delta", T)
        prodq = sb("prodq", T)
        denom = sb("denom", T)
        rcp = sb("rcp", T)
        t3 = sb("t3", T * 3)

        # ping-pong DMA sems: HWDGE queues complete out of order, so one
        # counting sem across tiles is unsound; even/odd tiles use separate
        # sems and at most two tiles are ever in flight.
        s_in = [
            ctx.enter_context(nc.semaphore("s_in0")),
            ctx.enter_context(nc.semaphore("s_in1")),
        ]
        s_out = [
            ctx.enter_context(nc.semaphore("s_out0")),
            ctx.enter_context(nc.semaphore("s_out1")),
        ]
        s_act = ctx.enter_context(nc.semaphore("s_act"))
        s_dve = ctx.enter_context(nc.semaphore("s_dve"))

        blk = ctx.enter_context(nc.Block())

        @blk.sync
        def _(sp):
            for i in range(n):
                j = i % 2
                if i >= 2:
                    # input slots j free: tile i-2's readers done
                    sp.wait_ge(s_act, am(i - 2, 2))   # zlin read zb, sig read ds
                    sp.wait_ge(s_dve, dm(i - 2, 7))   # wc read col, prob read pf
                sp.dma_start(out=zb[j][:], in_=zb_d[:, bass.ts(i, TK)]).then_inc(
                    s_in[j], 16
                )
                sp.dma_start(out=ds[j][:], in_=ds_d[:, bass.ts(i, TK)]).then_inc(
                    s_in[j], 16
                )
                sp.dma_start(out=pf[j][:], in_=pf_d[:, bass.ts(i, TK)]).then_inc(
                    s_in[j], 16
                )
                sp.dma_start(out=col[j][:], in_=pc_d[:, bass.ts(i, TK * 3)]).then_inc(
                    s_in[j], 16
                )
                if i >= 1:
                    sp.wait_ge(s_dve, dm(i - 1, 12))  # rgb written
                    sp.wait_ge(s_act, am(i - 1, 9))   # alpha written
                    sp.dma_start(
                        out=out_d[:, bass.ts(i - 1, T * 4)], in_=ot[(i - 1) % 2][:]
                    ).then_inc(s_out[(i - 1) % 2], 16)
            sp.wait_ge(s_dve, dm(n - 1, 12))
            sp.wait_ge(s_act, am(n - 1, 9))
            sp.dma_start(
                out=out_d[:, bass.ts(n - 1, T * 4)], in_=ot[(n - 1) % 2][:]
            ).then_inc(s_out[(n - 1) % 2], 16)
            sp.wait_ge(s_out[0], 16 * ((n + 1) // 2))
            sp.wait_ge(s_out[1], 16 * (n // 2))

        @blk.scalar
        def _(act):
            for i in range(n):
                j = i % 2
                act.wait_ge(s_in[j], 64 * (i // 2 + 1))
                if i >= 1:
                    act.wait_ge(s_dve, dm(i - 1, 6))  # w read prob(sig), zd read zinv(zlin)
                act.activation(sig[:], ds[j][:], Act.Sigmoid, scale=-1.0 / SIGMA)\
                    .then_inc(s_act, 1)                                   # 1 sig
                act.activation(
                    zlin[:], zb[j][:], Act.Copy,
                    bias=ZFAR / (ZFAR - ZNEAR), scale=-1.0 / (ZFAR - ZNEAR),
                ).then_inc(s_act, 1)                                      # 2 zlin
                act.wait_ge(s_dve, dm(i, 1))          # prob ready (in sig)
                act.activation(lnq[:], sig[:], Act.Ln, bias=1.0, scale=-1.0)\
                    .then_inc(s_act, 1)                                   # 3 lnq
                act.wait_ge(s_dve, dm(i, 5))          # zd ready
                act.activation(zd[:], zd[:], Act.Exp, scale=1.0 / GAMMA)\
                    .then_inc(s_act, 1)                                   # 4 ex
                act.activation(
                    delta[:], zmax[:], Act.Exp, bias=EPS / GAMMA, scale=-1.0 / GAMMA
                ).then_inc(s_act, 1)                                      # 5 delta
                act.activation(prodq[:], qsum[:], Act.Exp).then_inc(s_act, 1)  # 6 prodq
                act.drain()
                act.wait_ge(s_dve, dm(i, 10))         # denom ready
                act.activation(denom[:], denom[:], Act.Ln).then_inc(s_act, 1)  # 7 lnd
                act.drain()
                act.activation(rcp[:], denom[:], Act.Exp, scale=-1.0)\
                    .then_inc(s_act, 1)                                   # 8 rcp
                if i >= 2:
                    act.wait_ge(s_out[j], 16 * ((i - 2) // 2 + 1))
                ot_v = ot[j][:].rearrange("p (t q) -> p t q", q=4)
                act.activation(
                    ot_v[:, :, 3:4], prodq[:].unsqueeze(2), Act.Copy,
                    bias=1.0, scale=-1.0,
                ).then_inc(s_act, 1)                                      # 9 alpha

        @blk.vector
        def _(dve):
            for i in range(n):
                j = i % 2
                dve.wait_ge(s_in[j], 64 * (i // 2 + 1))
                dve.wait_ge(s_act, am(i, 2))
                dve.scalar_tensor_tensor(
                    out=sig[:], in0=pf[j][:], scalar=0.0, in1=sig[:],
                    op0=Alu.is_ge, op1=Alu.mult,
                ).then_inc(s_dve, 1)                                      # 1 prob
                dve.drain()
                dve.scalar_tensor_tensor(
                    out=zlin[:], in0=pf[j][:], scalar=0.0, in1=zlin[:],
                    op0=Alu.is_ge, op1=Alu.mult,
                ).then_inc(s_dve, 1)                                      # 2 zinv
                dve.drain()
                zinv_v = zlin[:].rearrange("p (t k) -> p t k", k=K)
                dve.tensor_reduce(
                    out=zmax[:], in_=zinv_v, op=Alu.max, axis=Ax.X
                ).then_inc(s_dve, 1)                                      # 3 zmax
                dve.drain()
                dve.wait_ge(s_act, am(i, 3))
                dve.tensor_reduce(
                    out=qsum[:],
                    in_=lnq[:].rearrange("p (t k) -> p t k", k=K),
                    op=Alu.add, axis=Ax.X,
                ).then_inc(s_dve, 1)                                      # 4 qsum
                dve.tensor_tensor(
                    out=zd[:].rearrange("p (t k) -> p t k", k=K),
                    in0=zinv_v,
                    in1=zmax[:].unsqueeze(2).broadcast_to((P, T, K)),
                    op=Alu.subtract,
                ).then_inc(s_dve, 1)                                      # 5 zd
                dve.wait_ge(s_act, am(i, 4))
                dve.tensor_tensor(
                    out=w[:], in0=sig[:], in1=zd[:], op=Alu.mult
                ).then_inc(s_dve, 1)                                      # 6 w
                dve.drain()
                wc_v = wc[:].rearrange("p (t c k) -> p t c k", c=3, k=K)
                dve.tensor_tensor(
                    out=wc_v,
                    in0=w[:].rearrange("p (t k) -> p t k", k=K)
                        .unsqueeze(2).broadcast_to((P, T, 3, K)),
                    in1=col[j][:].rearrange("p (t k c) -> p t c k", k=K, c=3),
                    op=Alu.mult,
                ).then_inc(s_dve, 1)                                      # 7 wc
                dve.drain()
                dve.tensor_reduce(
                    out=wsum[:],
                    in_=w[:].rearrange("p (t k) -> p t k", k=K),
                    op=Alu.add, axis=Ax.X,
                ).then_inc(s_dve, 1)                                      # 8 wsum
                dve.drain()
                csum_v = csum[:].rearrange("p (t c) -> p t c", c=3)
                dve.tensor_reduce(
                    out=csum_v, in_=wc_v, op=Alu.add, axis=Ax.X
                ).then_inc(s_dve, 1)                                      # 9 csum
                dve.drain()
                dve.wait_ge(s_act, am(i, 5))
                dve.tensor_tensor(
                    out=denom[:], in0=wsum[:], in1=delta[:], op=Alu.add
                ).then_inc(s_dve, 1)                                      # 10 denom
                t3_v = t3[:].rearrange("p (t c) -> p t c", c=3)
                dve.tensor_tensor(
                    out=t3_v,
                    in0=csum_v,
                    in1=delta[:].unsqueeze(2).broadcast_to((P, T, 3)),
                    op=Alu.add,
                ).then_inc(s_dve, 1)                                      # 11 t3
                dve.drain()
                if i >= 2:
                    # out-DMA of tile i-2 (same slot, same parity) drained
                    dve.wait_ge(s_out[j], 16 * ((i - 2) // 2 + 1))
                dve.wait_ge(s_act, am(i, 8))
                ot_v = ot[j][:].rearrange("p (t q) -> p t q", q=4)
                dve.tensor_tensor(
                    out=ot_v[:, :, 0:3],
                    in0=t3_v,
                    in1=rcp[:].unsqueeze(2).broadcast_to((P, T, 3)),
                    op=Alu.mult,
                ).then_inc(s_dve, 1)                                      # 12 rgb

    return nc


_CACHE = {}


def _get_program(rows=2048, T=256):
    key = (rows, T)
    if key not in _CACHE:
        _CACHE[key] = build_program(rows, T)
    return _CACHE[key]


def _run(pixel_colors, zbuf, dists, pix_to_face, trace=False):
    from concourse.bass_utils import run_bass_kernel_spmd

    N, H, W, Kk = zbuf.shape
    assert (N, H, W, Kk) == (8, 512, 512, 8), (N, H, W, Kk)
    rows = H * W // P  # 2048

    nc = _get_program(rows=rows, T=256)

    pc = np.ascontiguousarray(np.asarray(pixel_colors, dtype=np.float32))
    zb = np.ascontiguousarray(np.asarray(zbuf, dtype=np.float32))
    ds = np.ascontiguousarray(np.asarray(dists, dtype=np.float32))
    pf = np.ascontiguousarray(np.asarray(pix_to_face, dtype=np.int32))

    in_maps = []
    for i in range(N_CORES):
        in_maps.append(
            {
                "zbuf": zb[i].reshape(P, rows * K),
                "dists": ds[i].reshape(P, rows * K),
                "pix_to_face": pf[i].reshape(P, rows * K),
                "pixel_colors": pc[i].reshape(P, rows * K * 3),
            }
        )

    res = run_bass_kernel_spmd(
        nc, in_maps, core_ids=list(range(N_CORES)), trace=trace
    )
    out = np.stack(
        [res.results[i]["out"].reshape(H, W, 4) for i in range(N_CORES)], axis=0
    )
    return out, res


def kernel(pixel_colors, zbuf, dists, pix_to_face):
    out, _ = _run(pixel_colors, zbuf, dists, pix_to_face, trace=False)
    return out


# revision 14
# speedup vs baseline: 1.1013x; 1.1013x over previous
"""Trainium2 Bass kernel for softmax RGB blend (pytorch3d NoLightShader).

Full inputs (N=8, H=512, W=512, K=8) are sharded batch-wise across 8
NeuronCores (one batch image per core); the blend is purely per-pixel so no
cross-core communication is needed.

Math per pixel (K faces):
    mask_k  = pix_to_face_k >= 0
    prob_k  = sigmoid(-dists_k / SIGMA) * mask_k
    alpha   = 1 - prod_k(1 - prob_k)        (via exp(sum ln(1 - prob_k)))
    z_k     = (ZFAR - zbuf_k) / (ZFAR - ZNEAR) * mask_k
    zmax    = max_k z_k                     (EPS clamp dropped: only matters
                                             for all-masked pixels, where the
                                             result is unchanged)
    w_k     = prob_k * exp((z_k - zmax) / GAMMA)
    delta   = exp((EPS - zmax) / GAMMA)
    denom   = sum_k w_k + delta
    rgb_c   = (sum_k w_k * color_kc + delta) / denom    (background = 1,1,1)
    out     = [rgb, alpha]

Raw-bass pipeline (Tile's multi-wait instructions don't compile on this
walrus, so waits are explicit single-sem instructions):
    SP  (sync) : HWDGE DMAs in/out, double-buffered input slots
    ACT (scalar): sigmoid, z-linearize, ln(1-prob), exp(zd/g), delta, prod(q)
    DVE (vector): mask, mask applies, the four K-reductions, w, w*c,
                  denom, reciprocal, rgb/alpha finalize
Per-tile op indices give deterministic semaphore thresholds; see marks below.
"""

import sys
from contextlib import ExitStack

import numpy as np

if "/opt/trn_rl_repo" not in sys.path:
    sys.path.insert(0, "/opt/trn_rl_repo")

SIGMA = 1e-4
GAMMA = 1e-4
ZNEAR = 1.0
ZFAR = 100.0
EPS = 1e-10

P = 128
K = 8
N_CORES = 8

# per-tile op counts (sem increments per tile per engine)
N_ACT = 9   # sig, zlin, lnq, ex, delta, prodq, lnd, rcp, alpha
N_DVE = 12  # prob, zinv, zmax, qsum, zd, w, wc, wsum, csum, denom, t3, rgb


def build_program(rows, T):
    import concourse.bass as bass
    from concourse import mybir

    dt = mybir.dt
    f32 = dt.float32
    Alu = mybir.AluOpType
    Act = mybir.ActivationFunctionType
    Ax = mybir.AxisListType

    assert rows % T == 0
    n = rows // T
    TK = T * K

    nc = bass.Bass()

    zb_d = nc.dram_tensor("zbuf", [P, rows * K], f32, kind="ExternalInput")
    ds_d = nc.dram_tensor("dists", [P, rows * K], f32, kind="ExternalInput")
    pf_d = nc.dram_tensor("pix_to_face", [P, rows * K], dt.int32, kind="ExternalInput")
    pc_d = nc.dram_tensor("pixel_colors", [P, rows * K * 3], f32, kind="ExternalInput")
    out_d = nc.dram_tensor("out", [P, rows * 4], f32, kind="ExternalOutput")

    # const AP for the delta bias (EPS/GAMMA); framework pre-registers 0.0/1.0
    cbias = nc.alloc_sbuf_tensor("c_epsg", [P, 1], f32)
    nc.gpsimd.memset(cbias.ap(), EPS / GAMMA)
    nc.const_aps.aps[(f32, EPS / GAMMA)] = cbias.ap()
    nc.all_engine_barrier()

    # marks: value of the engine sem after op `idx` (1-based) of tile i
    am = lambda i, idx: i * N_ACT + idx
    dm = lambda i, idx: i * N_DVE + idx

    with ExitStack() as ctx:
        sb = lambda name, w: ctx.enter_context(nc.sbuf_tensor(name, [P, w], f32))
        # double-buffered input slots
        zb = [sb(f"zb{j}", TK) for j in range(2)]
        ds = [sb(f"ds{j}", TK) for j in range(2)]
        pf = [
            ctx.enter_context(nc.sbuf_tensor(f"pf{j}", [P, TK], dt.int32))
            for j in range(2)
        ]
        col = [sb(f"col{j}", TK * 3) for j in range(2)]
        ot = [sb(f"ot{j}", T * 4) for j in range(2)]
        # single-buffered intermediates (safety proven by the sem chains)
        sig = sb("sig", TK)      # becomes prob in place
        zlin = sb("zlin", TK)    # becomes zinv in place
        lnq = sb("lnq", TK)
        zd = sb("zd", TK)        # becomes ex in place
        w = sb("w", TK)
        wc = sb("wc", TK * 3)
        zmax = sb("zmax", T)
        qsum = sb("qsum", T)
        wsum = sb("wsum", T)
        csum = sb("csum", T * 3)
        delta = sb("delta", T)
        prodq = sb("prodq", T)
        denom = sb("denom", T)
        rcp = sb("rcp", T)
        t3 = sb("t3", T * 3)

        # ping-pong DMA sems: HWDGE queues complete out of order, so one
        # counting sem across tiles is unsound; even/odd tiles use separate
        # sems and at most two tiles are ever in flight.
        s_in = [
            ctx.enter_context(nc.semaphore("s_in0")),
            ctx.enter_context(nc.semaphore("s_in1")),
        ]
        s_out = [
            ctx.enter_context(nc.semaphore("s_out0")),
            ctx.enter_context(nc.semaphore("s_out1")),
        ]
        s_act = ctx.enter_context(nc.semaphore("s_act"))
        s_dve = ctx.enter_context(nc.semaphore("s_dve"))

        blk = ctx.enter_context(nc.Block())

        @blk.sync
        def _(sp):
            for i in range(n):
                j = i % 2
                if i >= 2:
                    # input slots j free: tile i-2's readers done
                    sp.wait_ge(s_act, am(i - 2, 2))   # zlin read zb, sig read ds
                    sp.wait_ge(s_dve, dm(i - 2, 9))   # wc read col, prob read pf
                sp.dma_start(out=zb[j][:], in_=zb_d[:, bass.ts(i, TK)]).then_inc(
                    s_in[j], 16
                )
                sp.dma_start(out=ds[j][:], in_=ds_d[:, bass.ts(i, TK)]).then_inc(
                    s_in[j], 16
                )
                sp.dma_start(out=pf[j][:], in_=pf_d[:, bass.ts(i, TK)]).then_inc(
                    s_in[j], 16
                )
                sp.dma_start(out=col[j][:], in_=pc_d[:, bass.ts(i, TK * 3)]).then_inc(
                    s_in[j], 16
                )
                if i >= 1:
                    sp.wait_ge(s_dve, dm(i - 1, 12))  # rgb written
                    sp.wait_ge(s_act, am(i - 1, 9))   # alpha written
                    sp.dma_start(
                        out=out_d[:, bass.ts(i - 1, T * 4)], in_=ot[(i - 1) % 2][:]
                    ).then_inc(s_out[(i - 1) % 2], 16)
            sp.wait_ge(s_dve, dm(n - 1, 12))
            sp.wait_ge(s_act, am(n - 1, 9))
            sp.dma_start(
                out=out_d[:, bass.ts(n - 1, T * 4)], in_=ot[(n - 1) % 2][:]
            ).then_inc(s_out[(n - 1) % 2], 16)
            sp.wait_ge(s_out[0], 16 * ((n + 1) // 2))
            sp.wait_ge(s_out[1], 16 * (n // 2))

        @blk.scalar
        def _(act):
            for i in range(n):
                j = i % 2
                act.wait_ge(s_in[j], 64 * (i // 2 + 1))
                if i >= 1:
                    act.wait_ge(s_dve, dm(i - 1, 6))  # w read prob(sig), zd read zinv(zlin)
                act.activation(sig[:], ds[j][:], Act.Sigmoid, scale=-1.0 / SIGMA)\
                    .then_inc(s_act, 1)                                   # 1 sig
                act.activation(
                    zlin[:], zb[j][:], Act.Copy,
                    bias=ZFAR / (ZFAR - ZNEAR), scale=-1.0 / (ZFAR - ZNEAR),
                ).then_inc(s_act, 1)                                      # 2 zlin
                act.wait_ge(s_dve, dm(i, 1))          # prob ready (in sig)
                act.activation(lnq[:], sig[:], Act.Ln, bias=1.0, scale=-1.0)\
                    .then_inc(s_act, 1)                                   # 3 lnq
                act.wait_ge(s_dve, dm(i, 5))          # zd ready
                act.activation(zd[:], zd[:], Act.Exp, scale=1.0 / GAMMA)\
                    .then_inc(s_act, 1)                                   # 4 ex
                act.activation(
                    delta[:], zmax[:], Act.Exp, bias=EPS / GAMMA, scale=-1.0 / GAMMA
                ).then_inc(s_act, 1)                                      # 5 delta
                act.activation(prodq[:], qsum[:], Act.Exp).then_inc(s_act, 1)  # 6 prodq
                act.drain()
                act.wait_ge(s_dve, dm(i, 8))          # denom ready
                act.activation(denom[:], denom[:], Act.Ln).then_inc(s_act, 1)  # 7 lnd
                act.drain()
                act.activation(rcp[:], denom[:], Act.Exp, scale=-1.0)\
                    .then_inc(s_act, 1)                                   # 8 rcp
                if i >= 2:
                    act.wait_ge(s_out[j], 16 * ((i - 2) // 2 + 1))
                ot_v = ot[j][:].rearrange("p (t q) -> p t q", q=4)
                act.activation(
                    ot_v[:, :, 3:4], prodq[:].unsqueeze(2), Act.Copy,
                    bias=1.0, scale=-1.0,
                ).then_inc(s_act, 1)                                      # 9 alpha

        @blk.vector
        def _(dve):
            for i in range(n):
                j = i % 2
                dve.wait_ge(s_in[j], 64 * (i // 2 + 1))
                dve.wait_ge(s_act, am(i, 2))
                dve.scalar_tensor_tensor(
                    out=sig[:], in0=pf[j][:], scalar=0.0, in1=sig[:],
                    op0=Alu.is_ge, op1=Alu.mult,
                ).then_inc(s_dve, 1)                                      # 1 prob
                dve.drain()
                dve.scalar_tensor_tensor(
                    out=zlin[:], in0=pf[j][:], scalar=0.0, in1=zlin[:],
                    op0=Alu.is_ge, op1=Alu.mult,
                ).then_inc(s_dve, 1)                                      # 2 zinv
                dve.drain()
                zinv_v = zlin[:].rearrange("p (t k) -> p t k", k=K)
                dve.tensor_reduce(
                    out=zmax[:], in_=zinv_v, op=Alu.max, axis=Ax.X
                ).then_inc(s_dve, 1)                                      # 3 zmax
                dve.drain()
                dve.wait_ge(s_act, am(i, 3))
                dve.tensor_reduce(
                    out=qsum[:],
                    in_=lnq[:].rearrange("p (t k) -> p t k", k=K),
                    op=Alu.add, axis=Ax.X,
                ).then_inc(s_dve, 1)                                      # 4 qsum
                dve.tensor_tensor(
                    out=zd[:].rearrange("p (t k) -> p t k", k=K),
                    in0=zinv_v,
                    in1=zmax[:].unsqueeze(2).broadcast_to((P, T, K)),
                    op=Alu.subtract,
                ).then_inc(s_dve, 1)                                      # 5 zd
                dve.wait_ge(s_act, am(i, 4))
                dve.tensor_tensor(
                    out=w[:], in0=sig[:], in1=zd[:], op=Alu.mult
                ).then_inc(s_dve, 1)                                      # 6 w
                dve.drain()
                dve.tensor_reduce(
                    out=wsum[:],
                    in_=w[:].rearrange("p (t k) -> p t k", k=K),
                    op=Alu.add, axis=Ax.X,
                ).then_inc(s_dve, 1)                                      # 7 wsum
                dve.drain()
                dve.wait_ge(s_act, am(i, 5))
                dve.tensor_tensor(
                    out=denom[:], in0=wsum[:], in1=delta[:], op=Alu.add
                ).then_inc(s_dve, 1)                                      # 8 denom
                wc_v = wc[:].rearrange("p (t c k) -> p t c k", c=3, k=K)
                dve.tensor_tensor(
                    out=wc_v,
                    in0=w[:].rearrange("p (t k) -> p t k", k=K)
                        .unsqueeze(2).broadcast_to((P, T, 3, K)),
                    in1=col[j][:].rearrange("p (t k c) -> p t c k", k=K, c=3),
                    op=Alu.mult,
                ).then_inc(s_dve, 1)                                      # 9 wc
                dve.drain()
                csum_v = csum[:].rearrange("p (t c) -> p t c", c=3)
                dve.tensor_reduce(
                    out=csum_v, in_=wc_v, op=Alu.add, axis=Ax.X
                ).then_inc(s_dve, 1)                                      # 10 csum
                dve.drain()
                t3_v = t3[:].rearrange("p (t c) -> p t c", c=3)
                dve.tensor_tensor(
                    out=t3_v,
                    in0=csum_v,
                    in1=delta[:].unsqueeze(2).broadcast_to((P, T, 3)),
                    op=Alu.add,
                ).then_inc(s_dve, 1)                                      # 11 t3
                dve.drain()
                if i >= 2:
                    # out-DMA of tile i-2 (same slot, same parity) drained
                    dve.wait_ge(s_out[j], 16 * ((i - 2) // 2 + 1))
                dve.wait_ge(s_act, am(i, 8))
                ot_v = ot[j][:].rearrange("p (t q) -> p t q", q=4)
                dve.tensor_tensor(
                    out=ot_v[:, :, 0:3],
                    in0=t3_v,
                    in1=rcp[:].unsqueeze(2).broadcast_to((P, T, 3)),
                    op=Alu.mult,
                ).then_inc(s_dve, 1)                                      # 12 rgb

    return nc


_CACHE = {}


def _get_program(rows=2048, T=256):
    key = (rows, T)
    if key not in _CACHE:
        _CACHE[key] = build_program(rows, T)
    return _CACHE[key]


def _run(pixel_colors, zbuf, dists, pix_to_face, trace=False):
    from concourse.bass_utils import run_bass_kernel_spmd

    N, H, W, Kk = zbuf.shape
    assert (N, H, W, Kk) == (8, 512, 512, 8), (N, H, W, Kk)
    rows = H * W // P  # 2048

    nc = _get_program(rows=rows, T=256)

    pc = np.ascontiguousarray(np.asarray(pixel_colors, dtype=np.float32))
    zb = np.ascontiguousarray(np.asarray(zbuf, dtype=np.float32))
    ds = np.ascontiguousarray(np.asarray(dists, dtype=np.float32))
    pf = np.ascontiguousarray(np.asarray(pix_to_face, dtype=np.int32))

    in_maps = []
    for i in range(N_CORES):
        in_maps.append(
            {
                "zbuf": zb[i].reshape(P, rows * K),
                "dists": ds[i].reshape(P, rows * K),
                "pix_to_face": pf[i].reshape(P, rows * K),
                "pixel_colors": pc[i].reshape(P, rows * K * 3),
            }
        )

    res = run_bass_kernel_spmd(
        nc, in_maps, core_ids=list(range(N_CORES)), trace=trace
    )
    out = np.stack(
        [res.results[i]["out"].reshape(H, W, 4) for i in range(N_CORES)], axis=0
    )
    return out, res


def kernel(pixel_colors, zbuf, dists, pix_to_face):
    out, _ = _run(pixel_colors, zbuf, dists, pix_to_face, trace=False)
    return out


# revision 16
# speedup vs baseline: 1.1405x; 1.0356x over previous
"""Trainium2 Bass kernel for softmax RGB blend (pytorch3d NoLightShader).

Full inputs (N=8, H=512, W=512, K=8) are sharded batch-wise across 8
NeuronCores (one batch image per core); the blend is purely per-pixel so no
cross-core communication is needed.

Math per pixel (K faces):
    mask_k  = pix_to_face_k >= 0
    prob_k  = sigmoid(-dists_k / SIGMA) * mask_k
    alpha   = 1 - prod_k(1 - prob_k)        (via exp(sum ln(1 - prob_k)))
    z_k     = (ZFAR - zbuf_k) / (ZFAR - ZNEAR) * mask_k
    zmax    = max_k z_k                     (EPS clamp dropped: only matters
                                             for all-masked pixels, where the
                                             result is unchanged)
    w_k     = prob_k * exp((z_k - zmax) / GAMMA)
    delta   = exp((EPS - zmax) / GAMMA)
    denom   = sum_k w_k + delta
    rgb_c   = (sum_k w_k * color_kc + delta) / denom    (background = 1,1,1)
    out     = [rgb, alpha]

Raw-bass pipeline (Tile's multi-wait instructions don't compile on this
walrus, so waits are explicit single-sem instructions):
    SP  (sync) : HWDGE DMAs in/out, double-buffered input slots
    ACT (scalar): sigmoid, z-linearize, ln(1-prob), exp(zd/g), delta, prod(q)
    DVE (vector): mask, mask applies, the four K-reductions, w, w*c,
                  denom, reciprocal, rgb/alpha finalize
Per-tile op indices give deterministic semaphore thresholds; see marks below.
"""

import sys
from contextlib import ExitStack

import numpy as np

if "/opt/trn_rl_repo" not in sys.path:
    sys.path.insert(0, "/opt/trn_rl_repo")

SIGMA = 1e-4
GAMMA = 1e-4
ZNEAR = 1.0
ZFAR = 100.0
EPS = 1e-10

P = 128
K = 8
N_CORES = 8

# per-tile op counts (sem increments per tile per engine)
N_ACT = 9   # sig, zlin, lnq, ex, delta, prodq, lnd, rcp, alpha
N_DVE = 11  # prob, zinv, zmax, qsum, w, wsum, denom, wc, csum, t3, rgb
N_GP = 1    # zd


def build_program(rows, T):
    import concourse.bass as bass
    from concourse import mybir

    dt = mybir.dt
    f32 = dt.float32
    Alu = mybir.AluOpType
    Act = mybir.ActivationFunctionType
    Ax = mybir.AxisListType

    assert rows % T == 0
    n = rows // T
    TK = T * K

    nc = bass.Bass()

    zb_d = nc.dram_tensor("zbuf", [P, rows * K], f32, kind="ExternalInput")
    ds_d = nc.dram_tensor("dists", [P, rows * K], f32, kind="ExternalInput")
    pf_d = nc.dram_tensor("pix_to_face", [P, rows * K], dt.int32, kind="ExternalInput")
    pc_d = nc.dram_tensor("pixel_colors", [P, rows * K * 3], f32, kind="ExternalInput")
    out_d = nc.dram_tensor("out", [P, rows * 4], f32, kind="ExternalOutput")

    # const AP for the delta bias (EPS/GAMMA); framework pre-registers 0.0/1.0
    cbias = nc.alloc_sbuf_tensor("c_epsg", [P, 1], f32)
    nc.gpsimd.memset(cbias.ap(), EPS / GAMMA)
    nc.const_aps.aps[(f32, EPS / GAMMA)] = cbias.ap()
    nc.all_engine_barrier()

    # marks: value of the engine sem after op `idx` (1-based) of tile i
    am = lambda i, idx: i * N_ACT + idx
    dm = lambda i, idx: i * N_DVE + idx

    with ExitStack() as ctx:
        sb = lambda name, w: ctx.enter_context(nc.sbuf_tensor(name, [P, w], f32))
        # double-buffered input slots
        zb = [sb(f"zb{j}", TK) for j in range(2)]
        ds = [sb(f"ds{j}", TK) for j in range(2)]
        pf = [
            ctx.enter_context(nc.sbuf_tensor(f"pf{j}", [P, TK], dt.int32))
            for j in range(2)
        ]
        col = [sb(f"col{j}", TK * 3) for j in range(2)]
        ot = [sb(f"ot{j}", T * 4) for j in range(2)]
        # single-buffered intermediates (safety proven by the sem chains)
        sig = sb("sig", TK)      # becomes prob in place
        zlin = sb("zlin", TK)    # becomes zinv in place
        lnq = sb("lnq", TK)
        zd = sb("zd", TK)        # becomes ex in place
        w = sb("w", TK)
        wc = sb("wc", TK * 3)
        zmax = sb("zmax", T)
        qsum = sb("qsum", T)
        wsum = sb("wsum", T)
        csum = sb("csum", T * 3)
        delta = sb("delta", T)
        prodq = sb("prodq", T)
        denom = sb("denom", T)
        rcp = sb("rcp", T)
        t3 = sb("t3", T * 3)

        # ping-pong DMA sems: HWDGE queues complete out of order, so one
        # counting sem across tiles is unsound; even/odd tiles use separate
        # sems and at most two tiles are ever in flight.
        s_in = [
            ctx.enter_context(nc.semaphore("s_in0")),
            ctx.enter_context(nc.semaphore("s_in1")),
        ]
        s_out = [
            ctx.enter_context(nc.semaphore("s_out0")),
            ctx.enter_context(nc.semaphore("s_out1")),
        ]
        s_act = ctx.enter_context(nc.semaphore("s_act"))
        s_dve = ctx.enter_context(nc.semaphore("s_dve"))
        s_gp = ctx.enter_context(nc.semaphore("s_gp"))

        blk = ctx.enter_context(nc.Block())

        @blk.sync
        def _(sp):
            for i in range(n):
                j = i % 2
                if i >= 2:
                    # input slots j free: tile i-2's readers done
                    sp.wait_ge(s_act, am(i - 2, 2))   # zlin read zb, sig read ds
                    sp.wait_ge(s_dve, dm(i - 2, 8))   # wc read col, prob read pf
                sp.dma_start(out=zb[j][:], in_=zb_d[:, bass.ts(i, TK)]).then_inc(
                    s_in[j], 16
                )
                sp.dma_start(out=ds[j][:], in_=ds_d[:, bass.ts(i, TK)]).then_inc(
                    s_in[j], 16
                )
                sp.dma_start(out=pf[j][:], in_=pf_d[:, bass.ts(i, TK)]).then_inc(
                    s_in[j], 16
                )
                sp.dma_start(out=col[j][:], in_=pc_d[:, bass.ts(i, TK * 3)]).then_inc(
                    s_in[j], 16
                )
                if i >= 1:
                    sp.wait_ge(s_dve, dm(i - 1, 11))  # rgb written
                    sp.wait_ge(s_act, am(i - 1, 9))   # alpha written
                    sp.dma_start(
                        out=out_d[:, bass.ts(i - 1, T * 4)], in_=ot[(i - 1) % 2][:]
                    ).then_inc(s_out[(i - 1) % 2], 16)
            sp.wait_ge(s_dve, dm(n - 1, 11))
            sp.wait_ge(s_act, am(n - 1, 9))
            sp.dma_start(
                out=out_d[:, bass.ts(n - 1, T * 4)], in_=ot[(n - 1) % 2][:]
            ).then_inc(s_out[(n - 1) % 2], 16)
            sp.wait_ge(s_out[0], 16 * ((n + 1) // 2))
            sp.wait_ge(s_out[1], 16 * (n // 2))

        @blk.scalar
        def _(act):
            for i in range(n):
                j = i % 2
                act.wait_ge(s_in[j], 64 * (i // 2 + 1))
                if i >= 1:
                    act.wait_ge(s_dve, dm(i - 1, 5))  # w read prob(sig)
                act.activation(sig[:], ds[j][:], Act.Sigmoid, scale=-1.0 / SIGMA)\
                    .then_inc(s_act, 1)                                   # 1 sig
                act.activation(
                    zlin[:], zb[j][:], Act.Copy,
                    bias=ZFAR / (ZFAR - ZNEAR), scale=-1.0 / (ZFAR - ZNEAR),
                ).then_inc(s_act, 1)                                      # 2 zlin
                act.wait_ge(s_dve, dm(i, 1))          # prob ready (in sig)
                act.activation(lnq[:], sig[:], Act.Ln, bias=1.0, scale=-1.0)\
                    .then_inc(s_act, 1)                                   # 3 lnq
                act.wait_ge(s_gp, i + 1)              # zd ready (gpsimd)
                act.activation(zd[:], zd[:], Act.Exp, scale=1.0 / GAMMA)\
                    .then_inc(s_act, 1)                                   # 4 ex
                act.activation(
                    delta[:], zmax[:], Act.Exp, bias=EPS / GAMMA, scale=-1.0 / GAMMA
                ).then_inc(s_act, 1)                                      # 5 delta
                act.wait_ge(s_dve, dm(i, 4))          # qsum ready
                act.activation(prodq[:], qsum[:], Act.Exp).then_inc(s_act, 1)  # 6 prodq
                act.drain()
                act.wait_ge(s_dve, dm(i, 7))          # denom ready
                act.activation(denom[:], denom[:], Act.Ln).then_inc(s_act, 1)  # 7 lnd
                act.drain()
                act.activation(rcp[:], denom[:], Act.Exp, scale=-1.0)\
                    .then_inc(s_act, 1)                                   # 8 rcp
                if i >= 2:
                    act.wait_ge(s_out[j], 16 * ((i - 2) // 2 + 1))
                ot_v = ot[j][:].rearrange("p (t q) -> p t q", q=4)
                act.activation(
                    ot_v[:, :, 3:4], prodq[:].unsqueeze(2), Act.Copy,
                    bias=1.0, scale=-1.0,
                ).then_inc(s_act, 1)                                      # 9 alpha

        @blk.gpsimd
        def _(gp):
            for i in range(n):
                gp.wait_ge(s_dve, dm(i, 3))           # zinv + zmax ready
                zinv_v = zlin[:].rearrange("p (t k) -> p t k", k=K)
                gp.tensor_tensor(
                    out=zd[:].rearrange("p (t k) -> p t k", k=K),
                    in0=zinv_v,
                    in1=zmax[:].unsqueeze(2).broadcast_to((P, T, K)),
                    op=Alu.subtract,
                ).then_inc(s_gp, 1)

        @blk.vector
        def _(dve):
            for i in range(n):
                j = i % 2
                dve.wait_ge(s_in[j], 64 * (i // 2 + 1))
                dve.wait_ge(s_act, am(i, 2))
                dve.scalar_tensor_tensor(
                    out=sig[:], in0=pf[j][:], scalar=0.0, in1=sig[:],
                    op0=Alu.is_ge, op1=Alu.mult,
                ).then_inc(s_dve, 1)                                      # 1 prob
                dve.drain()
                dve.scalar_tensor_tensor(
                    out=zlin[:], in0=pf[j][:], scalar=0.0, in1=zlin[:],
                    op0=Alu.is_ge, op1=Alu.mult,
                ).then_inc(s_dve, 1)                                      # 2 zinv
                dve.drain()
                zinv_v = zlin[:].rearrange("p (t k) -> p t k", k=K)
                dve.tensor_reduce(
                    out=zmax[:], in_=zinv_v, op=Alu.max, axis=Ax.X
                ).then_inc(s_dve, 1)                                      # 3 zmax
                dve.drain()
                dve.wait_ge(s_act, am(i, 3))
                dve.tensor_reduce(
                    out=qsum[:],
                    in_=lnq[:].rearrange("p (t k) -> p t k", k=K),
                    op=Alu.add, axis=Ax.X,
                ).then_inc(s_dve, 1)                                      # 4 qsum
                dve.wait_ge(s_act, am(i, 4))
                dve.tensor_tensor(
                    out=w[:], in0=sig[:], in1=zd[:], op=Alu.mult
                ).then_inc(s_dve, 1)                                      # 5 w
                dve.drain()
                dve.tensor_reduce(
                    out=wsum[:],
                    in_=w[:].rearrange("p (t k) -> p t k", k=K),
                    op=Alu.add, axis=Ax.X,
                ).then_inc(s_dve, 1)                                      # 6 wsum
                dve.drain()
                dve.wait_ge(s_act, am(i, 5))
                dve.tensor_tensor(
                    out=denom[:], in0=wsum[:], in1=delta[:], op=Alu.add
                ).then_inc(s_dve, 1)                                      # 7 denom
                wc_v = wc[:].rearrange("p (t c k) -> p t c k", c=3, k=K)
                dve.tensor_tensor(
                    out=wc_v,
                    in0=w[:].rearrange("p (t k) -> p t k", k=K)
                        .unsqueeze(2).broadcast_to((P, T, 3, K)),
                    in1=col[j][:].rearrange("p (t k c) -> p t c k", k=K, c=3),
                    op=Alu.mult,
                ).then_inc(s_dve, 1)                                      # 8 wc
                dve.drain()
                csum_v = csum[:].rearrange("p (t c) -> p t c", c=3)
                dve.tensor_reduce(
                    out=csum_v, in_=wc_v, op=Alu.add, axis=Ax.X
                ).then_inc(s_dve, 1)                                      # 9 csum
                dve.drain()
                t3_v = t3[:].rearrange("p (t c) -> p t c", c=3)
                dve.tensor_tensor(
                    out=t3_v,
                    in0=csum_v,
                    in1=delta[:].unsqueeze(2).broadcast_to((P, T, 3)),
                    op=Alu.add,
                ).then_inc(s_dve, 1)                                      # 10 t3
                dve.drain()
                if i >= 2:
                    # out-DMA of tile i-2 (same slot, same parity) drained
                    dve.wait_ge(s_out[j], 16 * ((i - 2) // 2 + 1))
                dve.wait_ge(s_act, am(i, 8))
                ot_v = ot[j][:].rearrange("p (t q) -> p t q", q=4)
                dve.tensor_tensor(
                    out=ot_v[:, :, 0:3],
                    in0=t3_v,
                    in1=rcp[:].unsqueeze(2).broadcast_to((P, T, 3)),
                    op=Alu.mult,
                ).then_inc(s_dve, 1)                                      # 11 rgb

    return nc


_CACHE = {}


def _get_program(rows=2048, T=256):
    key = (rows, T)
    if key not in _CACHE:
        _CACHE[key] = build_program(rows, T)
    return _CACHE[key]


def _run(pixel_colors, zbuf, dists, pix_to_face, trace=False):
    from concourse.bass_utils import run_bass_kernel_spmd

    N, H, W, Kk = zbuf.shape
    assert (N, H, W, Kk) == (8, 512, 512, 8), (N, H, W, Kk)
    rows = H * W // P  # 2048

    nc = _get_program(rows=rows, T=256)

    pc = np.ascontiguousarray(np.asarray(pixel_colors, dtype=np.float32))
    zb = np.ascontiguousarray(np.asarray(zbuf, dtype=np.float32))
    ds = np.ascontiguousarray(np.asarray(dists, dtype=np.float32))
    pf = np.ascontiguousarray(np.asarray(pix_to_face, dtype=np.int32))

    in_maps = []
    for i in range(N_CORES):
        in_maps.append(
            {
                "zbuf": zb[i].reshape(P, rows * K),
                "dists": ds[i].reshape(P, rows * K),
                "pix_to_face": pf[i].reshape(P, rows * K),
                "pixel_colors": pc[i].reshape(P, rows * K * 3),
            }
        )

    res = run_bass_kernel_spmd(
        nc, in_maps, core_ids=list(range(N_CORES)), trace=trace
    )
    out = np.stack(
        [res.results[i]["out"].reshape(H, W, 4) for i in range(N_CORES)], axis=0
    )
    return out, res


def kernel(pixel_colors, zbuf, dists, pix_to_face):
    out, _ = _run(pixel_colors, zbuf, dists, pix_to_face, trace=False)
    return out
